# revision 32
# baseline (speedup 1.0000x reference)
"""Causal multi-head attention kernel for Trainium2 (8 NeuronCores), v4.

Problem: B=2, H=16, S=2048, D=64 causal attention (softmax over last axis).
Sharding: 32 (batch, head) pairs split 4-per-core across 8 cores; each core
computes its heads independently (no collectives).

v4 design (~84us vs v3's ~93us; measured component walls: QK-only 15us,
QK+PV PE-only ~68us, exp ~78 engine-us + masks/copies ~19 over ACT+DVE):
  - QK: head PAIRS advance through k-blocks together, the two heads' QK
    matmuls (contraction d=64) emitted back-to-back with lhsT base
    partitions 0/64 so the PE row-tiles them into concurrent array
    halves; global 2-strip lookahead across pass/pair boundaries.
  - exact-causal strip widths: a strip (pass pa, k-block kb) covers
    q in [max(512*pa, 128*kb), 512*pa+512) -- no 512-aligned sub-diagonal
    waste in QK, exp, or PV (v3 computed+exp'd dead columns).
  - PV flipped to V-stationary: stationary = vg[:, kb, :] = [128k, 65]
    (V-block columns || ones), moving = pt[:, h, :w] (the exp'd scores,
    k on partitions, q on free).  One matmul per (strip, head) streaming
    w<=512 columns instead of v3's per-q_sub [128,128]-stationary matmuls
    (LDW-bound at ~128 cols load per 65 streamed).  Accumulates over kb
    into a [65, 512] PSUM acc per (pass, head): rows 0..63 =
    unnormalized out^T, row 64 = softmax denominator.
  - no on-device normalize: acc is copied (ACT/DVE, greedy-balanced) to
    SBUF bf16 and DMA'd out as [65, 512]; the host divides rows 0..63 by
    row 64 (same single bf16 quantization v3's normalized output had).
  - exp split ACT (exact, (FD+280)/1.2 ns) / DVE (Schraudolph one-op
    tensor_scalar -> int16 bitcast bf16, (FD+115)/0.96 ns, ~1.8% rms)
    by width-aware greedy load balance; intra-block causal mask (tri
    multiply) on DVE inline after exp.
  - PSUM exactly full: 3 st slots x 2 banks + 2 accs x 1 bank = 8 banks.
  - input DMAs hoisted to rep top, split across BOTH HWDGE rings (kt/vg
    on SP, qt on ACT) to halve per-ring FIFO serialization; outputs on
    the gpsimd (SWDGE) queue.
  Scheduling experiments that did NOT pay off on HW (kept out): pair-
  interleaving, PV lagged behind more QK lookahead, strip-pair batching
  (helped PE-only by ~15% but coupled worse in full), K=64-split PV
  (runtime INTERNAL fault with M=65 outputs).
"""

import sys

if "/opt/trn_rl_repo" not in sys.path:
    sys.path.insert(0, "/opt/trn_rl_repo")

import numpy as np
import ml_dtypes

B, H, S, D = 2, 16, 2048, 64
N_CORES = 8
HEADS_PER_CORE = (B * H) // N_CORES  # 4
KB = S // 128  # 16 k-blocks per head
NPASS = 4  # q passes per head pair (512 q each)

_BF16 = ml_dtypes.bfloat16

_built = {}

MODE = "full"
_MODE_FLAGS = {
    "full": "dqepn",
    "qk_noio": "q",
    "exp_pure": "e",
    "qe_pure": "qe",
    "qep_pure": "qep",
    "qp_pure": "qp",   # QK + PV with const pt (no exp): pure PE throughput
    "no_out": "dqep",
}

# Fraction of exp pieces computed on DVE (Schraudolph) instead of ACT.
# DVE strip = (120 + FD/2)/0.96 ns (2x mode), ACT strip = (FD + 352)/1.2 ns.
DVE_EXP = 0.6
# Fraction of output copies on DVE (rest on ACT/scalar).
DVE_COPY = 0.5

# Schraudolph constants: exp(s*0.125) ~= bf16_bits(round(s*A + B))
_SCH_A = 0.125 * 1.4426950408889634 * 128.0
_SCH_B = 16256.0 - 4.8

# perf probes (wrong math, only for qp_pure attribution):
PROBE_PV_KB0 = False    # reuse vg[:, 0, :] stationary for every PV MM
PROBE_PV_NOACC = False  # start=stop=True on every PV MM (no accum groups)


def _pass_plan(causal, pa):
    """For pass pa (q in [512*pa, 512*pa+512)): list of (kb, qlo, qhi)
    exact-causal strips, qlo = max(512*pa, 128*kb)."""
    q_lo = 512 * pa
    q_hi = q_lo + 512
    plan = []
    for kb in range(KB):
        if causal and 128 * kb >= q_hi:
            continue
        qlo = max(q_lo, 128 * kb) if causal else q_lo
        plan.append((kb, qlo, q_hi))
    return plan


def _emit(tc, nc, mybir, qt, kt, vg, tri, o2, causal, reps=1):
    from contextlib import ExitStack, nullcontext

    flags = _MODE_FLAGS[MODE]
    f32 = mybir.dt.float32
    bf = mybir.dt.bfloat16
    i16 = mybir.dt.int16
    Exp = mybir.ActivationFunctionType.Exp
    Mult = mybir.AluOpType.mult
    Add = mybir.AluOpType.add

    with ExitStack() as ctx:
        const = ctx.enter_context(tc.tile_pool(name="const", bufs=1))
        qk = ctx.enter_context(tc.tile_pool(name="qk", bufs=3))
        vpool = ctx.enter_context(tc.tile_pool(name="vp", bufs=3))
        ptp = ctx.enter_context(tc.tile_pool(name="ptp", bufs=4))
        outp = ctx.enter_context(tc.tile_pool(name="outp", bufs=6))
        stp = ctx.enter_context(tc.tile_pool(name="stp", bufs=3, space="PSUM"))
        accp = ctx.enter_context(tc.tile_pool(name="accp", bufs=1, space="PSUM"))

        tri_t = const.tile([128, 128], bf, name="tri_t")
        nc.sync.dma_start(tri_t[:, :], tri[:, :])

        warm = const.tile([128, 1], f32, name="warm")
        nc.vector.memset(warm[:, :], 0.0)
        nc.scalar.activation(warm[:, :], warm[:, :], Exp)

        if "d" not in flags:
            qt_c = const.tile([128, S], bf, name="qt_const")
            kt_c = const.tile([128, S], bf, name="kt_const")
            vg_c = const.tile([128, KB, 65], bf, name="vg_const")
            nc.vector.memset(qt_c[:, :], 0.0)
            nc.vector.memset(kt_c[:, :], 0.0)
            nc.vector.memset(vg_c[:, :, :], 0.0)
        if "q" not in flags and "e" in flags:
            cpsum = ctx.enter_context(
                tc.tile_pool(name="cpsum", bufs=1, space="PSUM"))
            st_cA = cpsum.tile([128, 2, 512], f32, name="st_constA")
            st_cB = cpsum.tile([128, 2, 512], f32, name="st_constB")
            nc.vector.memset(st_cA[:, :, :], 0.0)
            nc.vector.memset(st_cB[:, :, :], 0.0)
            st_consts = [st_cA, st_cB]
            st_ctr = [0]
        if "p" in flags and "e" not in flags:
            pt_c = const.tile([128, 2, 512], bf, name="pt_const")
            nc.vector.memset(pt_c[:, :, :], 0.5)

        dve_acc = [0.0]

        def use_dve():
            if DVE_EXP <= 0.0 or "e" not in flags:
                return False
            dve_acc[0] += DVE_EXP
            if dve_acc[0] >= 1.0:
                dve_acc[0] -= 1.0
                return True
            return False

        cp_acc = [0.0]

        def copy_on_dve():
            cp_acc[0] += DVE_COPY
            if cp_acc[0] >= 1.0:
                cp_acc[0] -= 1.0
                return True
            return False

        with (tc.For_i(0, reps, 1) if reps > 1 else nullcontext()):
          rep = 0
          tiles = {}
          for p in range(HEADS_PER_CORE // 2):
              # all input loads at the top of the rep on the SP queue (kept
              # free of output DMAs so the next rep's loads prefetch early);
              # chunked so pair0's first QK unblocks quickly
              if "d" in flags:
                  qt_t = qk.tile([128, S], bf, tag="qt", name=f"qt_{p}")
                  kt_t = qk.tile([128, S], bf, tag="kt", name=f"kt_{p}")
                  vg_t0 = vpool.tile([128, KB, 65], bf, tag="vg0",
                                     name=f"vg_{p}_0")
                  vg_t1 = vpool.tile([128, KB, 65], bf, tag="vg1",
                                     name=f"vg_{p}_1")
                  # inputs split across the two HWDGE rings (SP + ACT) so
                  # the per-ring FIFO serialization halves: kt/vg on SP,
                  # qt on the ACT ring
                  nc.sync.dma_start(kt_t[:, :128], kt[p][:, :128])
                  nc.scalar.dma_start(qt_t[:, :512], qt[p][:, :512])
                  nc.sync.dma_start(kt_t[:, 128:], kt[p][:, 128:])
                  nc.scalar.dma_start(qt_t[:, 512:1024], qt[p][:, 512:1024])
                  nc.sync.dma_start(vg_t0[:, :, :], vg[2 * p])
                  nc.sync.dma_start(vg_t1[:, :, :], vg[2 * p + 1])
                  nc.scalar.dma_start(qt_t[:, 1024:], qt[p][:, 1024:])
              else:
                  qt_t, kt_t = qt_c, kt_c
                  vg_t0 = vg_t1 = vg_c
              tiles[p] = (qt_t, kt_t, (vg_t0, vg_t1))

          # global strip list across (pair, pass) with exact-causal widths
          gsteps = []
          for p in range(HEADS_PER_CORE // 2):
              for pa in range(NPASS):
                  plan = _pass_plan(causal, pa)
                  for t, (kb, qlo, qhi) in enumerate(plan):
                      gsteps.append(dict(
                          p=p, pa=pa, kb=kb, qlo=qlo, qhi=qhi,
                          first=(t == 0), last=(t == len(plan) - 1)))
          ng = len(gsteps)

          # width-aware greedy engine assignment for exp: balance measured
          # per-strip costs ACT ~(FD/1.2 + 233ns), DVE ~(FD/0.96 + 79ns).
          # The diagonal-strip mask (2x ~127ns) always lands on DVE (it is
          # a DVE-only op) but is emitted at batch start, long after its
          # exp completed, so it never stalls either queue.
          act_load, dve_load = 0.0, 0.0
          exp_on_dve = []
          for g in gsteps:
              w2 = 2 * (g["qhi"] - g["qlo"])
              is_diag = causal and g["qlo"] <= 128 * g["kb"] < g["qhi"]
              c_act = w2 / 1.2 + 233.0
              c_dve = w2 / 0.96 + 79.0
              if is_diag:
                  dve_load += 254.0
              if dve_load + c_dve <= act_load + c_act:
                  exp_on_dve.append(True)
                  dve_load += c_dve
              else:
                  exp_on_dve.append(False)
                  act_load += c_act
          # copies: ~720ns ACT / ~658ns DVE each, balance onto both
          cp_eng = []
          for _ in range(4 * NPASS):
              if dve_load + 658.0 <= act_load + 720.0:
                  cp_eng.append("dve")
                  dve_load += 658.0
              else:
                  cp_eng.append("act")
                  act_load += 720.0

          def emit_qk_g(gi):
              g = gsteps[gi]
              qt_g, kt_g, _ = tiles[g["p"]]
              if "q" not in flags:
                  if "e" in flags:
                      st_ctr[0] += 1
                      return st_consts[st_ctr[0] % 2]
                  return None
              st = stp.tile([128, 2, 512], f32, tag="st",
                            name=f"st_{g['p']}_{g['pa']}_{g['kb']}")
              w = g["qhi"] - g["qlo"]
              for h in range(2):
                  po = 64 * h
                  nc.tensor.matmul(
                      st[:, h, :w],
                      lhsT=kt_g[po:po + 64,
                                g["kb"] * 128:(g["kb"] + 1) * 128],
                      rhs=qt_g[po:po + 64, g["qlo"]:g["qhi"]],
                      start=True, stop=True,
                  )
              return st

          def emit_exp_g(gi, st):
              if "e" not in flags:
                  return None
              g = gsteps[gi]
              w = g["qhi"] - g["qlo"]
              dcol = 128 * g["kb"]
              pt = ptp.tile([128, 2, 512], bf, tag="pt",
                            name=f"pt_{g['p']}_{g['pa']}_{g['kb']}")
              if exp_on_dve[gi]:
                  nc.vector.tensor_scalar(
                      pt[:, :, :w].bitcast(i16), st[:, :, :w],
                      _SCH_A, _SCH_B, Mult, Add)
              else:
                  nc.scalar.activation(pt[:, :, :w], st[:, :, :w],
                                       Exp, scale=0.125)
              if causal and g["qlo"] <= dcol < g["qhi"]:
                  dg = dcol - g["qlo"]
                  for h in range(2):
                      nc.vector.tensor_mul(pt[:, h, dg:dg + 128],
                                           pt[:, h, dg:dg + 128],
                                           tri_t[:, :])
              return pt

          seg_accs = [None]  # accs of the seg currently accumulating
          cp_ctr = [0]

          def emit_pv_g(gi, pt):
              if "p" not in flags:
                  return
              if pt is None:
                  pt = pt_c
              g = gsteps[gi]
              p, pa = g["p"], g["pa"]
              if g["first"]:
                  seg_accs[0] = [
                      accp.tile([65, 512], f32, tag=f"acc{h}",
                                name=f"acc_{p}_{pa}_{h}")
                      for h in range(2)]
              accs = seg_accs[0]
              w = g["qhi"] - g["qlo"]
              s0 = g["qlo"] - 512 * pa
              _, _, vg_ts = tiles[p]
              kb_st = 0 if PROBE_PV_KB0 else g["kb"]
              for h in range(2):
                  nc.tensor.matmul(
                      accs[h][:, s0:s0 + w],
                      lhsT=vg_ts[h][:, kb_st, :],
                      rhs=pt[:, h, :w],
                      start=g["first"],
                      stop=g["last"],
                  )

          def emit_out(gi):
              g = gsteps[gi]
              if not (g["last"] and "n" in flags and "p" in flags):
                  return
              p, pa = g["p"], g["pa"]
              accs = seg_accs[0]
              # copy accs to SBUF bf16 and DMA out (host normalizes)
              for h in range(2):
                  ct = outp.tile([65, 512], bf, tag=f"ct{h}",
                                 name=f"ct_{p}_{pa}_{h}")
                  if cp_eng[cp_ctr[0]] == "dve":
                      nc.vector.tensor_copy(ct[:, :], accs[h][:, :])
                  else:
                      nc.scalar.copy(ct[:, :], accs[h][:, :])
                  cp_ctr[0] += 1
                  h_g = 2 * p + h
                  nc.gpsimd.dma_start(o2[h_g, :, pa, :], ct[:, :])

          # global walk in strip PAIRS: PE sees [QK(b+3), QK(b+4)] then
          # [PV(b) x2, PV(b+1) x2].  Batching QKs (tile config (64,128))
          # apart from PVs ((128,128)) halves PE row-config transitions
          # (PE-only: 58us batched vs 68us per-strip); lookahead 3-4 fits
          # the 3-slot st ring so the st WAR never gates the PE beyond
          # what PV(b+1) already needs.  Masks stay inline after exp.
          pts = {}
          for j in range(min(3, ng)):
              pts[j] = emit_exp_g(j, emit_qk_g(j))
          for b in range(0, ng, 2):
              for j in (b + 3, b + 4):
                  if j < ng:
                      pts[j] = emit_exp_g(j, emit_qk_g(j))
              for j in (b, b + 1):
                  if j < ng:
                      emit_pv_g(j, pts.pop(j))
                      emit_out(j)


def build_nc(causal=True, reps=1):
    key = ("nc7", causal, reps, MODE, DVE_EXP, DVE_COPY,
           PROBE_PV_KB0, PROBE_PV_NOACC)
    if key in _built:
        return _built[key]
    import concourse.bacc as bacc
    from concourse import mybir, tile

    nc = bacc.Bacc("TRN2", target_bir_lowering=False, debug=False,
                   num_devices=N_CORES)
    qt = nc.dram_tensor("qt", (HEADS_PER_CORE // 2, 128, S),
                        mybir.dt.bfloat16, kind="ExternalInput").ap()
    kt = nc.dram_tensor("kt", (HEADS_PER_CORE // 2, 128, S),
                        mybir.dt.bfloat16, kind="ExternalInput").ap()
    vg = nc.dram_tensor("vg", (HEADS_PER_CORE, 128, KB, 65),
                        mybir.dt.bfloat16, kind="ExternalInput").ap()
    tri = nc.dram_tensor("tri", (128, 128), mybir.dt.bfloat16,
                         kind="ExternalInput").ap()
    # output: per (head, pass): [65, 512] = unnormalized out^T || denom row
    o2 = nc.dram_tensor("o2", (HEADS_PER_CORE, 65, NPASS, 512),
                        mybir.dt.bfloat16, kind="ExternalOutput").ap()
    with tile.TileContext(nc) as tc:
        _emit(tc, nc, mybir, qt, kt, vg, tri, o2, causal, reps)
    nc.compile()
    _built[key] = nc
    return nc


def prep_inputs(Q, K, V):
    Qf = np.ascontiguousarray(Q, dtype=np.float32).reshape(B * H, S, D)
    Kf = np.ascontiguousarray(K, dtype=np.float32).reshape(B * H, S, D)
    Vf = np.ascontiguousarray(V, dtype=np.float32).reshape(B * H, S, D)

    Qt = np.ascontiguousarray(Qf.transpose(0, 2, 1)).astype(_BF16)
    Kt = np.ascontiguousarray(Kf.transpose(0, 2, 1)).astype(_BF16)

    Vb = Vf.astype(_BF16)
    vg_all = np.empty((B * H, 128, KB, 65), dtype=_BF16)
    vg_all[:, :, :, :64] = Vb.reshape(B * H, KB, 128, D).transpose(0, 2, 1, 3)
    vg_all[:, :, :, 64] = _BF16(1.0)

    tri_np = (np.tril(np.ones((128, 128), dtype=np.float32))
              .T.astype(_BF16))
    tri_np = np.ascontiguousarray(tri_np)

    in_maps = []
    for c in range(N_CORES):
        h0 = c * HEADS_PER_CORE
        qt_c = np.empty((HEADS_PER_CORE // 2, 128, S), dtype=_BF16)
        kt_c = np.empty((HEADS_PER_CORE // 2, 128, S), dtype=_BF16)
        for p in range(HEADS_PER_CORE // 2):
            qt_c[p, :64] = Qt[h0 + 2 * p]
            qt_c[p, 64:] = Qt[h0 + 2 * p + 1]
            kt_c[p, :64] = Kt[h0 + 2 * p]
            kt_c[p, 64:] = Kt[h0 + 2 * p + 1]
        in_maps.append({
            "qt": qt_c,
            "kt": kt_c,
            "vg": np.ascontiguousarray(vg_all[h0:h0 + HEADS_PER_CORE]),
            "tri": tri_np,
        })
    return in_maps


def _classify_mask(mask):
    m = np.asarray(mask).reshape(S, S)
    if not m.any():
        return "dense"
    if np.array_equal(m, np.triu(np.ones((S, S), dtype=bool), k=1)):
        return "causal"
    raise NotImplementedError("only causal or all-False masks supported")


def run_cores(in_maps, causal=True, reps=1, **kwargs):
    from concourse import bass_utils

    nc = build_nc(causal, reps)
    return bass_utils.run_bass_kernel_spmd(
        nc, in_maps, core_ids=list(range(N_CORES)), **kwargs
    )


def kernel(Q, K, V, mask):
    kind = _classify_mask(mask)
    in_maps = prep_inputs(Q, K, V)
    res = run_cores(in_maps, causal=(kind == "causal"))
    outs = []
    for r in res.results:
        o2 = np.asarray(r["o2"], dtype=np.float32)  # [4, 65, NPASS, 512]
        num = o2[:, :64, :, :]          # [4, 64, 4, 512] = out^T chunks
        den = o2[:, 64, :, :]           # [4, 4, 512] softmax denominators
        o = num.transpose(0, 2, 3, 1) / den[:, :, :, None]
        outs.append(o.reshape(HEADS_PER_CORE, S, D))
    out = np.concatenate(outs, axis=0)
    return np.ascontiguousarray(out.reshape(B, H, S, D), dtype=np.float32)


if __name__ == "__main__":
    rng = np.random.default_rng(0)
    Q = rng.standard_normal((B, H, S, D), dtype=np.float32)
    K = rng.standard_normal((B, H, S, D), dtype=np.float32)
    V = rng.standard_normal((B, H, S, D), dtype=np.float32)
    mask = np.triu(np.ones((S, S), dtype=bool), k=1)[None, None]
    out = kernel(Q, K, V, mask)
    print("out", out.shape, out.dtype)


# revision 34
# speedup vs baseline: 1.1327x; 1.1327x over previous
"""Causal multi-head attention kernel for Trainium2 (8 NeuronCores), v4.

Problem: B=2, H=16, S=2048, D=64 causal attention (softmax over last axis).
Sharding: 32 (batch, head) pairs split 4-per-core across 8 cores; each core
computes its heads independently (no collectives).

v4 design (~84us vs v3's ~93us; measured component walls: QK-only 15us,
QK+PV PE-only ~68us, exp ~78 engine-us + masks/copies ~19 over ACT+DVE):
  - QK: head PAIRS advance through k-blocks together, the two heads' QK
    matmuls (contraction d=64) emitted back-to-back with lhsT base
    partitions 0/64 so the PE row-tiles them into concurrent array
    halves; global 2-strip lookahead across pass/pair boundaries.
  - exact-causal strip widths: a strip (pass pa, k-block kb) covers
    q in [max(512*pa, 128*kb), 512*pa+512) -- no 512-aligned sub-diagonal
    waste in QK, exp, or PV (v3 computed+exp'd dead columns).
  - PV flipped to V-stationary: stationary = vg[:, kb, :] = [128k, 65]
    (V-block columns || ones), moving = pt[:, h, :w] (the exp'd scores,
    k on partitions, q on free).  One matmul per (strip, head) streaming
    w<=512 columns instead of v3's per-q_sub [128,128]-stationary matmuls
    (LDW-bound at ~128 cols load per 65 streamed).  Accumulates over kb
    into a [65, 512] PSUM acc per (pass, head): rows 0..63 =
    unnormalized out^T, row 64 = softmax denominator.
  - no on-device normalize: acc is copied (ACT/DVE, greedy-balanced) to
    SBUF bf16 and DMA'd out as [65, 512]; the host divides rows 0..63 by
    row 64 (same single bf16 quantization v3's normalized output had).
  - exp split ACT (exact, (FD+280)/1.2 ns) / DVE (Schraudolph one-op
    tensor_scalar -> int16 bitcast bf16, (FD+115)/0.96 ns, ~1.8% rms)
    by width-aware greedy load balance; intra-block causal mask (tri
    multiply) on DVE inline after exp.
  - PSUM exactly full: 3 st slots x 2 banks + 2 accs x 1 bank = 8 banks.
  - input DMAs hoisted to rep top, split across BOTH HWDGE rings (kt/vg
    on SP, qt on ACT) to halve per-ring FIFO serialization; outputs on
    the gpsimd (SWDGE) queue.
  Scheduling experiments that did NOT pay off on HW (kept out): pair-
  interleaving, PV lagged behind more QK lookahead, strip-pair batching
  (helped PE-only by ~15% but coupled worse in full), K=64-split PV
  (runtime INTERNAL fault with M=65 outputs).
"""

import sys

if "/opt/trn_rl_repo" not in sys.path:
    sys.path.insert(0, "/opt/trn_rl_repo")

import numpy as np
import ml_dtypes

B, H, S, D = 2, 16, 2048, 64
N_CORES = 8
HEADS_PER_CORE = (B * H) // N_CORES  # 4
KB = S // 128  # 16 k-blocks per head
NPASS = 4  # q passes per head pair (512 q each)

_BF16 = ml_dtypes.bfloat16

_built = {}

MODE = "full"
_MODE_FLAGS = {
    "full": "dqepn",
    "qk_noio": "q",
    "exp_pure": "e",
    "qe_pure": "qe",
    "qep_pure": "qep",
    "qp_pure": "qp",   # QK + PV with const pt (no exp): pure PE throughput
    "no_out": "dqep",
}

# Fraction of exp pieces computed on DVE (Schraudolph) instead of ACT.
# DVE strip = (120 + FD/2)/0.96 ns (2x mode), ACT strip = (FD + 352)/1.2 ns.
DVE_EXP = 0.6
# Fraction of output copies on DVE (rest on ACT/scalar).
DVE_COPY = 0.5

# Schraudolph constants: exp(s*0.125) ~= bf16_bits(round(s*A + B))
_SCH_A = 0.125 * 1.4426950408889634 * 128.0
_SCH_B = 16256.0 - 4.8

# perf probes (wrong math, only for qp_pure attribution):
PROBE_PV_KB0 = False    # reuse vg[:, 0, :] stationary for every PV MM
PROBE_PV_NOACC = False  # start=stop=True on every PV MM (no accum groups)


def _pass_plan(causal, pa):
    """For pass pa (q in [512*pa, 512*pa+512)): list of (kb, qlo, qhi)
    exact-causal strips, qlo = max(512*pa, 128*kb)."""
    q_lo = 512 * pa
    q_hi = q_lo + 512
    plan = []
    for kb in range(KB):
        if causal and 128 * kb >= q_hi:
            continue
        qlo = max(q_lo, 128 * kb) if causal else q_lo
        plan.append((kb, qlo, q_hi))
    return plan


def _emit(tc, nc, mybir, qt, kt, vg, tri, o2, causal, reps=1):
    from contextlib import ExitStack, nullcontext

    flags = _MODE_FLAGS[MODE]
    f32 = mybir.dt.float32
    bf = mybir.dt.bfloat16
    i16 = mybir.dt.int16
    Exp = mybir.ActivationFunctionType.Exp
    Mult = mybir.AluOpType.mult
    Add = mybir.AluOpType.add

    with ExitStack() as ctx:
        const = ctx.enter_context(tc.tile_pool(name="const", bufs=1))
        qk = ctx.enter_context(tc.tile_pool(name="qk", bufs=3))
        vpool = ctx.enter_context(tc.tile_pool(name="vp", bufs=3))
        ptp = ctx.enter_context(tc.tile_pool(name="ptp", bufs=4))
        outp = ctx.enter_context(tc.tile_pool(name="outp", bufs=6))
        stp = ctx.enter_context(tc.tile_pool(name="stp", bufs=3, space="PSUM"))
        accp = ctx.enter_context(tc.tile_pool(name="accp", bufs=1, space="PSUM"))

        tri_t = const.tile([128, 128], bf, name="tri_t")
        nc.sync.dma_start(tri_t[:, :], tri[:, :])

        warm = const.tile([128, 1], f32, name="warm")
        nc.vector.memset(warm[:, :], 0.0)
        nc.scalar.activation(warm[:, :], warm[:, :], Exp)

        if "d" not in flags:
            qt_c = const.tile([128, S], bf, name="qt_const")
            kt_c = const.tile([128, S], bf, name="kt_const")
            vg_c = const.tile([128, KB, 65], bf, name="vg_const")
            nc.vector.memset(qt_c[:, :], 0.0)
            nc.vector.memset(kt_c[:, :], 0.0)
            nc.vector.memset(vg_c[:, :, :], 0.0)
        if "q" not in flags and "e" in flags:
            cpsum = ctx.enter_context(
                tc.tile_pool(name="cpsum", bufs=1, space="PSUM"))
            st_cA = cpsum.tile([128, 2, 512], f32, name="st_constA")
            st_cB = cpsum.tile([128, 2, 512], f32, name="st_constB")
            nc.vector.memset(st_cA[:, :, :], 0.0)
            nc.vector.memset(st_cB[:, :, :], 0.0)
            st_consts = [st_cA, st_cB]
            st_ctr = [0]
        if "p" in flags and "e" not in flags:
            pt_c = const.tile([128, 2, 512], bf, name="pt_const")
            nc.vector.memset(pt_c[:, :, :], 0.5)

        dve_acc = [0.0]

        def use_dve():
            if DVE_EXP <= 0.0 or "e" not in flags:
                return False
            dve_acc[0] += DVE_EXP
            if dve_acc[0] >= 1.0:
                dve_acc[0] -= 1.0
                return True
            return False

        cp_acc = [0.0]

        def copy_on_dve():
            cp_acc[0] += DVE_COPY
            if cp_acc[0] >= 1.0:
                cp_acc[0] -= 1.0
                return True
            return False

        with (tc.For_i(0, reps, 1) if reps > 1 else nullcontext()):
          rep = 0
          tiles = {}
          for p in range(HEADS_PER_CORE // 2):
              # all input loads at the top of the rep on the SP queue (kept
              # free of output DMAs so the next rep's loads prefetch early);
              # chunked so pair0's first QK unblocks quickly
              if "d" in flags:
                  qt_t = qk.tile([128, S], bf, tag="qt", name=f"qt_{p}")
                  kt_t = qk.tile([128, S], bf, tag="kt", name=f"kt_{p}")
                  vg_t0 = vpool.tile([128, KB, 65], bf, tag="vg0",
                                     name=f"vg_{p}_0")
                  vg_t1 = vpool.tile([128, KB, 65], bf, tag="vg1",
                                     name=f"vg_{p}_1")
                  # inputs split across the two HWDGE rings (SP + ACT) so
                  # the per-ring FIFO serialization halves: kt/vg on SP,
                  # qt on the ACT ring
                  nc.sync.dma_start(kt_t[:, :128], kt[p][:, :128])
                  nc.scalar.dma_start(qt_t[:, :512], qt[p][:, :512])
                  nc.sync.dma_start(kt_t[:, 128:], kt[p][:, 128:])
                  nc.scalar.dma_start(qt_t[:, 512:1024], qt[p][:, 512:1024])
                  nc.sync.dma_start(vg_t0[:, :, :], vg[2 * p])
                  nc.sync.dma_start(vg_t1[:, :, :], vg[2 * p + 1])
                  nc.scalar.dma_start(qt_t[:, 1024:], qt[p][:, 1024:])
              else:
                  qt_t, kt_t = qt_c, kt_c
                  vg_t0 = vg_t1 = vg_c
              tiles[p] = (qt_t, kt_t, (vg_t0, vg_t1))

          # global strip list across (pair, pass) with exact-causal widths
          gsteps = []
          for p in range(HEADS_PER_CORE // 2):
              for pa in range(NPASS):
                  plan = _pass_plan(causal, pa)
                  for t, (kb, qlo, qhi) in enumerate(plan):
                      gsteps.append(dict(
                          p=p, pa=pa, kb=kb, qlo=qlo, qhi=qhi,
                          first=(t == 0), last=(t == len(plan) - 1)))
          ng = len(gsteps)

          # width-aware greedy engine assignment for exp: balance measured
          # per-strip costs ACT ~(FD/1.2 + 233ns), DVE ~(FD/0.96 + 79ns).
          # The diagonal-strip mask (2x ~127ns) always lands on DVE (it is
          # a DVE-only op) but is emitted at batch start, long after its
          # exp completed, so it never stalls either queue.
          act_load, dve_load = 0.0, 0.0
          exp_on_dve = []
          for g in gsteps:
              w2 = 2 * (g["qhi"] - g["qlo"])
              is_diag = causal and g["qlo"] <= 128 * g["kb"] < g["qhi"]
              c_act = w2 / 1.2 + 233.0
              c_dve = w2 / 0.96 + 79.0
              if is_diag:
                  dve_load += 254.0
              if dve_load + c_dve <= act_load + c_act:
                  exp_on_dve.append(True)
                  dve_load += c_dve
              else:
                  exp_on_dve.append(False)
                  act_load += c_act
          # copies: ~720ns ACT / ~658ns DVE each, balance onto both
          cp_eng = []
          for _ in range(4 * NPASS):
              if dve_load + 658.0 <= act_load + 720.0:
                  cp_eng.append("dve")
                  dve_load += 658.0
              else:
                  cp_eng.append("act")
                  act_load += 720.0

          def emit_qk_g(gi):
              g = gsteps[gi]
              qt_g, kt_g, _ = tiles[g["p"]]
              if "q" not in flags:
                  if "e" in flags:
                      st_ctr[0] += 1
                      return st_consts[st_ctr[0] % 2]
                  return None
              st = stp.tile([128, 2, 512], f32, tag="st",
                            name=f"st_{g['p']}_{g['pa']}_{g['kb']}")
              w = g["qhi"] - g["qlo"]
              for h in range(2):
                  po = 64 * h
                  nc.tensor.matmul(
                      st[:, h, :w],
                      lhsT=kt_g[po:po + 64,
                                g["kb"] * 128:(g["kb"] + 1) * 128],
                      rhs=qt_g[po:po + 64, g["qlo"]:g["qhi"]],
                      start=True, stop=True,
                  )
              return st

          def emit_exp_g(gi, st):
              if "e" not in flags:
                  return None
              g = gsteps[gi]
              w = g["qhi"] - g["qlo"]
              dcol = 128 * g["kb"]
              pt = ptp.tile([128, 2, 512], bf, tag="pt",
                            name=f"pt_{g['p']}_{g['pa']}_{g['kb']}")
              if exp_on_dve[gi]:
                  nc.vector.tensor_scalar(
                      pt[:, :, :w].bitcast(i16), st[:, :, :w],
                      _SCH_A, _SCH_B, Mult, Add)
              else:
                  nc.scalar.activation(pt[:, :, :w], st[:, :, :w],
                                       Exp, scale=0.125)
              if causal and g["qlo"] <= dcol < g["qhi"]:
                  dg = dcol - g["qlo"]
                  for h in range(2):
                      nc.vector.tensor_mul(pt[:, h, dg:dg + 128],
                                           pt[:, h, dg:dg + 128],
                                           tri_t[:, :])
              return pt

          seg_accs = [None]  # accs of the seg currently accumulating
          cp_ctr = [0]

          def emit_pv_g(gi, pt):
              if "p" not in flags:
                  return
              if pt is None:
                  pt = pt_c
              g = gsteps[gi]
              p, pa = g["p"], g["pa"]
              if g["first"]:
                  seg_accs[0] = [
                      accp.tile([65, 512], f32, tag=f"acc{h}",
                                name=f"acc_{p}_{pa}_{h}")
                      for h in range(2)]
              accs = seg_accs[0]
              w = g["qhi"] - g["qlo"]
              s0 = g["qlo"] - 512 * pa
              _, _, vg_ts = tiles[p]
              kb_st = 0 if PROBE_PV_KB0 else g["kb"]
              for h in range(2):
                  nc.tensor.matmul(
                      accs[h][:, s0:s0 + w],
                      lhsT=vg_ts[h][:, kb_st, :],
                      rhs=pt[:, h, :w],
                      start=g["first"],
                      stop=g["last"],
                  )

          seg_cts = [None]

          def emit_out(gi):
              # Split copy: columns [0:384] of a causal pass are fully
              # accumulated after strip kb=4pa+2 (column block j completes
              # at kb=4pa+j), so the bulk copy runs there, off the
              # critical path.  At the last strip only a [65,128] tail
              # copy + the DMA remain before the acc bank (ring bufs=1)
              # can be reused by the next seg -- the seg-boundary WAR
              # stall shrinks from ~700ns/copy to ~250ns.
              g = gsteps[gi]
              if not ("n" in flags and "p" in flags):
                  return
              p, pa = g["p"], g["pa"]
              accs = seg_accs[0]
              early = causal and g["kb"] == 4 * pa + 2
              if early:
                  seg_cts[0] = []
                  for h in range(2):
                      ct = outp.tile([65, 512], bf, tag=f"ct{h}",
                                     name=f"ct_{p}_{pa}_{h}")
                      seg_cts[0].append((ct, cp_eng[cp_ctr[0]]))
                      if cp_eng[cp_ctr[0]] == "dve":
                          nc.vector.tensor_copy(ct[:, :384],
                                                accs[h][:, :384])
                      else:
                          nc.scalar.copy(ct[:, :384], accs[h][:, :384])
                      cp_ctr[0] += 1
                  return
              if not g["last"]:
                  return
              for h in range(2):
                  if causal:
                      ct, eng = seg_cts[0][h]
                      if eng == "dve":
                          nc.vector.tensor_copy(ct[:, 384:],
                                                accs[h][:, 384:])
                      else:
                          nc.scalar.copy(ct[:, 384:], accs[h][:, 384:])
                  else:
                      ct = outp.tile([65, 512], bf, tag=f"ct{h}",
                                     name=f"ct_{p}_{pa}_{h}")
                      if cp_eng[cp_ctr[0]] == "dve":
                          nc.vector.tensor_copy(ct[:, :], accs[h][:, :])
                      else:
                          nc.scalar.copy(ct[:, :], accs[h][:, :])
                      cp_ctr[0] += 1
                  h_g = 2 * p + h
                  nc.gpsimd.dma_start(o2[h_g, :, pa, :], ct[:, :])

          # global walk (v4 structure, best measured): per strip the PE
          # sees [QK(g+2), PV(g)] with the 2-ahead QK window carried
          # across pass and pair boundaries; exp(g) runs while the PE
          # does QK(g+1)/QK(g+2)/PV(g-1).
          sts_q = [emit_qk_g(0)]
          if ng > 1:
              sts_q.append(emit_qk_g(1))
          for gi in range(ng):
              st = sts_q.pop(0)
              pt = emit_exp_g(gi, st)
              if gi + 2 < ng:
                  sts_q.append(emit_qk_g(gi + 2))
              emit_pv_g(gi, pt)
              emit_out(gi)


def build_nc(causal=True, reps=1):
    key = ("nc7", causal, reps, MODE, DVE_EXP, DVE_COPY,
           PROBE_PV_KB0, PROBE_PV_NOACC)
    if key in _built:
        return _built[key]
    import concourse.bacc as bacc
    from concourse import mybir, tile

    nc = bacc.Bacc("TRN2", target_bir_lowering=False, debug=False,
                   num_devices=N_CORES)
    qt = nc.dram_tensor("qt", (HEADS_PER_CORE // 2, 128, S),
                        mybir.dt.bfloat16, kind="ExternalInput").ap()
    kt = nc.dram_tensor("kt", (HEADS_PER_CORE // 2, 128, S),
                        mybir.dt.bfloat16, kind="ExternalInput").ap()
    vg = nc.dram_tensor("vg", (HEADS_PER_CORE, 128, KB, 65),
                        mybir.dt.bfloat16, kind="ExternalInput").ap()
    tri = nc.dram_tensor("tri", (128, 128), mybir.dt.bfloat16,
                         kind="ExternalInput").ap()
    # output: per (head, pass): [65, 512] = unnormalized out^T || denom row
    o2 = nc.dram_tensor("o2", (HEADS_PER_CORE, 65, NPASS, 512),
                        mybir.dt.bfloat16, kind="ExternalOutput").ap()
    with tile.TileContext(nc) as tc:
        _emit(tc, nc, mybir, qt, kt, vg, tri, o2, causal, reps)
    nc.compile()
    _built[key] = nc
    return nc


def prep_inputs(Q, K, V):
    Qf = np.ascontiguousarray(Q, dtype=np.float32).reshape(B * H, S, D)
    Kf = np.ascontiguousarray(K, dtype=np.float32).reshape(B * H, S, D)
    Vf = np.ascontiguousarray(V, dtype=np.float32).reshape(B * H, S, D)

    Qt = np.ascontiguousarray(Qf.transpose(0, 2, 1)).astype(_BF16)
    Kt = np.ascontiguousarray(Kf.transpose(0, 2, 1)).astype(_BF16)

    Vb = Vf.astype(_BF16)
    vg_all = np.empty((B * H, 128, KB, 65), dtype=_BF16)
    vg_all[:, :, :, :64] = Vb.reshape(B * H, KB, 128, D).transpose(0, 2, 1, 3)
    vg_all[:, :, :, 64] = _BF16(1.0)

    tri_np = (np.tril(np.ones((128, 128), dtype=np.float32))
              .T.astype(_BF16))
    tri_np = np.ascontiguousarray(tri_np)

    in_maps = []
    for c in range(N_CORES):
        h0 = c * HEADS_PER_CORE
        qt_c = np.empty((HEADS_PER_CORE // 2, 128, S), dtype=_BF16)
        kt_c = np.empty((HEADS_PER_CORE // 2, 128, S), dtype=_BF16)
        for p in range(HEADS_PER_CORE // 2):
            qt_c[p, :64] = Qt[h0 + 2 * p]
            qt_c[p, 64:] = Qt[h0 + 2 * p + 1]
            kt_c[p, :64] = Kt[h0 + 2 * p]
            kt_c[p, 64:] = Kt[h0 + 2 * p + 1]
        in_maps.append({
            "qt": qt_c,
            "kt": kt_c,
            "vg": np.ascontiguousarray(vg_all[h0:h0 + HEADS_PER_CORE]),
            "tri": tri_np,
        })
    return in_maps


def _classify_mask(mask):
    m = np.asarray(mask).reshape(S, S)
    if not m.any():
        return "dense"
    if np.array_equal(m, np.triu(np.ones((S, S), dtype=bool), k=1)):
        return "causal"
    raise NotImplementedError("only causal or all-False masks supported")


def run_cores(in_maps, causal=True, reps=1, **kwargs):
    from concourse import bass_utils

    nc = build_nc(causal, reps)
    return bass_utils.run_bass_kernel_spmd(
        nc, in_maps, core_ids=list(range(N_CORES)), **kwargs
    )


def kernel(Q, K, V, mask):
    kind = _classify_mask(mask)
    in_maps = prep_inputs(Q, K, V)
    res = run_cores(in_maps, causal=(kind == "causal"))
    outs = []
    for r in res.results:
        o2 = np.asarray(r["o2"], dtype=np.float32)  # [4, 65, NPASS, 512]
        num = o2[:, :64, :, :]          # [4, 64, 4, 512] = out^T chunks
        den = o2[:, 64, :, :]           # [4, 4, 512] softmax denominators
        o = num.transpose(0, 2, 3, 1) / den[:, :, :, None]
        outs.append(o.reshape(HEADS_PER_CORE, S, D))
    out = np.concatenate(outs, axis=0)
    return np.ascontiguousarray(out.reshape(B, H, S, D), dtype=np.float32)


if __name__ == "__main__":
    rng = np.random.default_rng(0)
    Q = rng.standard_normal((B, H, S, D), dtype=np.float32)
    K = rng.standard_normal((B, H, S, D), dtype=np.float32)
    V = rng.standard_normal((B, H, S, D), dtype=np.float32)
    mask = np.triu(np.ones((S, S), dtype=bool), k=1)[None, None]
    out = kernel(Q, K, V, mask)
    print("out", out.shape, out.dtype)
